# revision 8
# baseline (speedup 1.0000x reference)
"""Trainium2 Bass kernel for nn_CSCLoss: multi-scale bilinear point-sampling
cosine-consistency loss.

loss = 1 - mean_{pairs,(b,n)} <normalize(sample(feat_i, p_bn)), normalize(sample(feat_j, p_bn))>

Sharding: data-parallel over batch - 32 images -> 8 cores x 4 images; the
host sums the 8 per-core partial sums and applies the loss epilogue.

Per-core dataflow (v2). ap_gather cost = ~3us fixed per instruction plus a
small per-index marginal, so the design uses plain d=1 4-corner gathers and
minimizes gather COUNT, not index count:
 - 6 stream DMAs total (T2, T1, 4x T0-image), all [128, *] tiles holding
   both 128-channel chunks side by side; no shifted-B copies, no l1 HBM
   re-read: HBM traffic is exactly the 22 MB of inputs.
 - One 1024-idx gather per l2/l1 level, one 256-idx gather per l0 image
   (indices address [chunk, image, y0*W+x0] + corner offsets directly).
 - Index/weight math on partition 0 in wide fused DVE ops; int16 indices
   replicated to the 8 gpsimd core groups via a sync-queue DRAM round trip
   (HWDGE; the Pool queue runs ONLY ap_gather - no SWDGE ucode swaps).
 - Per-chunk channel sums (ones-matmul into PSUM) right after each V slice;
   l1/l2 norms + the (1,2) pair run early; only l0-dependent epilogue rides
   the tail.
"""

import sys
from contextlib import ExitStack

import numpy as np

if "/opt/trn_rl_repo" not in sys.path:
    sys.path.insert(0, "/opt/trn_rl_repo")

B, N, C = 32, 32, 256
LEVELS = [(64, 64), (32, 32), (16, 16)]  # (H, W)
N_CORES = 8
BL = B // N_CORES          # images per core
NPTS = BL * N              # 128 points per core
PAIRS = [(0, 1), (0, 2), (1, 2)]
EPS = 1e-12

_CACHE = {}


def _build_program():
    from concourse import bacc, bass, mybir, tile, library_config

    dt = mybir.dt
    AL = mybir.AluOpType
    F32 = dt.float32
    I16 = dt.int16
    I32 = dt.int32

    nc = bacc.Bacc("TRN2", target_bir_lowering=False, debug=False)

    feats = [
        nc.dram_tensor(f"feat{i}", [BL, C, H, W], F32, kind="ExternalInput")
        for i, (H, W) in enumerate(LEVELS)
    ]
    boxes = nc.dram_tensor("boxes", [BL, N, 4], F32, kind="ExternalInput")
    out = nc.dram_tensor("out", [1, 1], F32, kind="ExternalOutput")

    with tile.TileContext(nc) as tc, ExitStack() as ctx:
        pool = ctx.enter_context(tc.tile_pool(name="sbuf", bufs=1))
        pa = ctx.enter_context(tc.tile_pool(name="pa", bufs=1))
        pstream = ctx.enter_context(tc.tile_pool(name="stream", bufs=1))
        pwork = ctx.enter_context(tc.tile_pool(name="work", bufs=2))
        ppsum = ctx.enter_context(tc.tile_pool(name="psum", bufs=1, space="PSUM"))
        pdram = ctx.enter_context(tc.tile_pool(name="dram", bufs=1, space="DRAM"))

        nc.gpsimd.load_library(library_config.ap_gather)

        # warm-up: the first custom-op dispatch pays the Q7 ucode
        # installation; absorb it under the streaming with a dummy gather.
        dg_src = pool.tile([128, 4], F32, name="dg_src")
        nc.vector.memset(dg_src[:], 0.0)
        dg_idx = pool.tile([128, 1], I16, name="dg_idx")
        nc.vector.memset(dg_idx[:], 0)
        dg_out = pool.tile([128, 16], F32, name="dg_out")
        nc.gpsimd.ap_gather(
            out_ap=dg_out[:], in_ap=dg_src[:], idxs_ap=dg_idx[:],
            channels=128, num_elems=4, d=1, num_idxs=16,
        )

        # ---- boxes first on the sync queue ----
        bxr = pool.tile([1, BL * N * 4], F32)
        nc.sync.dma_start(
            out=bxr[:].rearrange("o (a f) -> o a f", a=BL),
            in_=boxes.rearrange("b n c -> b (n c)"),
        )

        # ---- stream tiles: [chunk | chunk] layouts, plain d=1 sources ----
        # T2 cols: b*512 + sec*256 + (y*16+x)   (b outer so DRAM dims merge)
        # T1 cols: b*2048 + sec*1024 + (y*32+x)
        # T0_u cols: sec*4096 + (y*64+x)
        T2 = pstream.tile([128, 2048], F32, name="T2")     # 8 KB/part
        T1 = pstream.tile([128, 8192], F32, name="T1")     # 32 KB/part
        T0 = [
            pstream.tile([128, 8192], F32, name=f"T0_{u}", tag="T0", bufs=2)
            for u in range(BL)
        ]
        fv2 = feats[2].rearrange("b (s c) h w -> c b s (h w)", s=2)
        nc.scalar.dma_start(
            out=T2[:].rearrange("c (b s p) -> c b s p", s=2, b=BL), in_=fv2
        )
        fv1 = feats[1].rearrange("b (s c) h w -> c b s (h w)", s=2)
        nc.scalar.dma_start(
            out=T1[:].rearrange("c (b s p) -> c b s p", s=2, b=BL), in_=fv1
        )
        fv0 = feats[0].rearrange("b (s c) h w -> b c s (h w)", s=2)
        for u in range(BL):
            nc.scalar.dma_start(
                out=T0[u][:].rearrange("c (s p) -> c s p", s=2), in_=fv0[u]
            )

        # ---- constants (DVE, no deps - run under the stream head) ----
        # per-level column layout on [1, 384]: cols li*128 + (b*32 + n)
        LSEG = lambda t, li: t[:, li * 128:(li + 1) * 128]
        WVf = pa.tile([1, 384], F32, name="WVf")    # W per level (y stride)
        WVi = pa.tile([1, 384], I16, name="WVi")
        for li, (H, W) in enumerate(LEVELS):
            nc.vector.memset(LSEG(WVf, li), float(W))
            nc.vector.memset(LSEG(WVi, li), W)
        OFFV = pa.tile([1, 384], F32, name="OFFV")  # per-image tile offset
        nc.vector.memset(LSEG(OFFV, 0), 0.0)
        for li in (1, 2):
            bstride = 2 * LEVELS[li][0] * LEVELS[li][1]  # both chunks
            ov = LSEG(OFFV, li).rearrange("o (b n) -> o b n", b=BL)
            for b in range(BL):
                nc.vector.memset(ov[:, b], float(b * bstride))
        ones1 = pool.tile([1, 128], F32, name="ones1")
        nc.vector.memset(ones1[:], 1.0)
        ones = pool.tile([128, 1], F32)
        nc.vector.memset(ones[:], 1.0)

        # ---- Phase A: per-point scalar math on partition 0 (DVE) ----
        # X-layout [1, 768]: col = li*256 + ax*128 + pt   (ax: 0=x, 1=y)
        PF = pa.tile([1, 768], F32, name="PF")
        cview = bxr[:].rearrange("o (pt c) -> o c pt", c=4)
        for li, (H, W) in enumerate(LEVELS):
            sl = slice(li * 256, (li + 1) * 256)
            pv = PF[:, sl].rearrange("o (ax pt) -> o ax pt", ax=2)
            # p = clip(c*(E-1), 0, E-1)
            nc.vector.tensor_scalar(
                out=pv, in0=cview[:, 0:2, :], scalar1=float(W - 1),
                scalar2=0.0, op0=AL.mult, op1=AL.max,
            )
            nc.vector.tensor_scalar_min(
                out=PF[:, sl], in0=PF[:, sl], scalar1=float(W - 1)
            )
        # e0 = clamp(floor(p), 0, E-2); floor via 16.16 fixed point (exact)
        PXS = pa.tile([1, 768], F32, name="PXS")
        nc.vector.tensor_scalar(
            out=PXS[:], in0=PF[:], scalar1=65536.0, scalar2=None, op0=AL.mult
        )
        IFX = pa.tile([1, 768], I32, name="IFX")
        nc.vector.tensor_copy(out=IFX[:], in_=PXS[:])
        nc.vector.tensor_scalar(
            out=IFX[:], in0=IFX[:], scalar1=16, scalar2=None,
            op0=AL.arith_shift_right,
        )
        E0F = pa.tile([1, 768], F32, name="E0F")
        nc.vector.tensor_copy(out=E0F[:], in_=IFX[:])
        for li, (H, W) in enumerate(LEVELS):
            sl = slice(li * 256, (li + 1) * 256)
            nc.vector.tensor_scalar_min(
                out=E0F[:, sl], in0=E0F[:, sl], scalar1=float(W - 2)
            )
        # W2 [1, 2*768]: col = w*768 + li*256 + ax*128 + pt  (w: 0=1-frac, 1=frac)
        W2 = pa.tile([1, 1536], F32, name="W2")
        nc.vector.tensor_tensor(
            out=W2[:, 768:1536], in0=PF[:], in1=E0F[:], op=AL.subtract
        )
        nc.vector.tensor_scalar(
            out=W2[:, 0:768], in0=W2[:, 768:1536], scalar1=-1.0, scalar2=1.0,
            op0=AL.mult, op1=AL.add,
        )
        # base = b_off + y0*W + x0  -> BI int16 [1, 384]
        E0v = E0F[:].rearrange("o (li ax pt) -> o li ax pt", li=3, ax=2)
        BF = pa.tile([1, 384], F32, name="BF")
        BFv = BF[:].rearrange("o (li pt) -> o li pt", li=3)
        nc.vector.tensor_tensor(
            out=BFv, in0=E0v[:, :, 1, :],
            in1=WVf[:].rearrange("o (li pt) -> o li pt", li=3), op=AL.mult
        )
        nc.vector.tensor_tensor(out=BFv, in0=BFv, in1=E0v[:, :, 0, :], op=AL.add)
        nc.vector.tensor_tensor(
            out=BFv, in0=BFv,
            in1=OFFV[:].rearrange("o (li pt) -> o li pt", li=3), op=AL.add
        )
        BI = pa.tile([1, 384], I16, name="BI")
        nc.vector.tensor_copy(out=BI[:], in_=BF[:])
        # BK [1, 4*384] i16: 4-corner bases, col = k*384 + li*128 + pt
        # dk(k) = (k//2)*W + (k%2)
        BK = pa.tile([1, 4 * 384], I16, name="BK")
        nc.vector.tensor_copy(out=BK[:, 0:384], in_=BI[:])
        nc.vector.tensor_scalar_add(out=BK[:, 384:768], in0=BI[:], scalar1=1)
        nc.vector.tensor_tensor(
            out=BK[:, 768:1152], in0=BI[:], in1=WVi[:], op=AL.add
        )
        nc.vector.tensor_scalar_add(
            out=BK[:, 1152:1536], in0=BK[:, 768:1152], scalar1=1
        )

        # ---- gather index rows: idx #m lives at wrapped [r=m%16, q=m//16] ----
        # srow2: l2, m = sec*512 + pt*4 + k            (Q=64)
        # srowB: l1 at q 0:64 (same m-scheme), l0 at q 64+u*16+sec*8+(n//4),
        #        m_local = sec*128 + n*4 + k per image (Q=128)
        srow2 = pa.tile([1, 1024], I16, name="srow2")
        srowB = pa.tile([1, 2048], I16, name="srowB")

        def bk_view(li):
            # [o, pm, k, pd] view of BK for level li; pt = pd*4 + pm
            return BK[:].rearrange(
                "o (k li pd pm) -> o li pm k pd", k=4, li=3, pd=32, pm=4
            )[:, li]

        def scatter_L(srow, q0, qtot, li, secoff):
            # srow flat col = r*qtot + q; r = (pt%4)*4 + k, q = q0 + sec*32 + pd
            sv = srow[:].rearrange(
                "o (pm k q) -> o pm k q", pm=4, k=4
            )
            for sec in range(2):
                ov = sv[:, :, :, q0 + sec * 32:q0 + sec * 32 + 32]
                nc.vector.tensor_scalar_add(
                    out=ov, in0=bk_view(li), scalar1=sec * secoff
                )

        scatter_L(srow2, 0, 64, 2, 256)
        # replicate l2 rows early: SBUF -> DRAM -> broadcast (sync HWDGE)
        widx2 = pool.tile([128, 64], I16, name="widx2")
        s2d = pdram.tile([16, 64], I16, name="s2d")
        nc.sync.dma_start(
            out=s2d[:], in_=srow2[:].rearrange("o (r q) -> o r q", r=16)
        )
        nc.sync.dma_start(
            out=widx2[:], in_=s2d[:].unsqueeze(0).broadcast_to([8, 16, 64])
        )

        scatter_L(srowB, 0, 128, 1, 1024)
        # l0: srowB flat col = r*128 + 64 + u*16 + sec*8 + nd; r = nm*4 + k
        sv0 = srowB[:].rearrange("o (nm k q) -> o nm k q", nm=4, k=4)
        b0v = BK[:].rearrange(
            "o (k li u nd nm) -> o li u nm k nd", k=4, li=3, u=BL, nd=8, nm=4
        )
        for u in range(BL):
            for sec in range(2):
                ov = sv0[:, :, :, 64 + u * 16 + sec * 8:64 + u * 16 + sec * 8 + 8]
                nc.vector.tensor_scalar_add(
                    out=ov, in0=b0v[:, 0, u], scalar1=sec * 4096
                )
        widxB = pool.tile([128, 128], I16, name="widxB")
        sBd = pdram.tile([16, 128], I16, name="sBd")
        nc.sync.dma_start(
            out=sBd[:], in_=srowB[:].rearrange("o (r q) -> o r q", r=16)
        )
        nc.sync.dma_start(
            out=widxB[:], in_=sBd[:].unsqueeze(0).broadcast_to([8, 16, 128])
        )

        # ---- lerp weights wrow [1, 1536] -> wb [128, 1536] ----
        # col = w0(level) + pt*4 + k, k = row*2 + j; weight = yw(row)*xw(j)
        # level regions: l2 at 0, l1 at 512, l0 at 1024 (pt = u*32+n)
        wrow = pa.tile([1, 1536], F32, name="wrow")
        for li, w0 in ((2, 0), (1, 512), (0, 1024)):
            wseg = wrow[:, w0:w0 + 512].rearrange(
                "o (pt row j) -> o pt row j", pt=128, row=2, j=2
            )
            for row in range(2):
                yv = W2[:, row * 768 + li * 256 + 128:row * 768 + li * 256 + 256]
                for j in range(2):
                    xv = W2[:, j * 768 + li * 256:j * 768 + li * 256 + 128]
                    nc.vector.tensor_tensor(
                        out=wseg[:, :, row, j], in0=yv, in1=xv, op=AL.mult
                    )
        wb_ps = ppsum.tile([128, 1536], F32, name="wb_ps")
        for i in range(3):
            nc.tensor.matmul(
                wb_ps[:, i * 512:(i + 1) * 512], ones1[:],
                wrow[:, i * 512:(i + 1) * 512], start=True, stop=True,
            )
        wb = pool.tile([128, 1536], F32, name="wb")
        nc.vector.tensor_copy(out=wb[:], in_=wb_ps[:])

        # ---- gathers + lerp + reduce + per-chunk channel sums ----
        V = pool.tile([128, 768], F32, name="V")

        ps_ss = ppsum.tile([1, 512], F32, name="ps_ss")    # ss2 | ss1
        ps_ss0 = ppsum.tile([1, 256], F32, name="ps_ss0")  # (u, sec, n)
        ps_d12 = ppsum.tile([1, 256], F32, name="ps_d12")
        ps_d01 = ppsum.tile([1, 256], F32, name="ps_d01")
        ps_d02 = ppsum.tile([1, 256], F32, name="ps_d02")

        def colsum(ps_slice, in0, in1, n, tag):
            prod = pwork.tile([128, 256], F32, name=f"prod{tag}", tag="prod",
                              bufs=1)
            nc.vector.tensor_tensor(
                out=prod[:, 0:n], in0=in0, in1=in1, op=AL.mult
            )
            nc.tensor.matmul(
                ps_slice, ones[:], prod[:, 0:n], start=True, stop=True
            )

        def gatherL(li, T, idxs, v0, w0, tag):
            og = pwork.tile([128, 1024], F32, name=f"og{tag}", tag="ogL",
                            bufs=2)
            nc.gpsimd.ap_gather(
                out_ap=og[:], in_ap=T[:], idxs_ap=idxs,
                channels=128, num_elems=T.shape[1], d=1, num_idxs=1024,
            )
            for sec in range(2):
                nc.vector.tensor_tensor(
                    out=og[:, sec * 512:(sec + 1) * 512],
                    in0=og[:, sec * 512:(sec + 1) * 512],
                    in1=wb[:, w0:w0 + 512], op=AL.mult,
                )
            nc.vector.tensor_reduce(
                out=V[:, v0:v0 + 256],
                in_=og[:].rearrange("c (n f) -> c n f", f=4),
                axis=mybir.AxisListType.X, op=AL.add,
            )
            colsum(ps_ss[:, v0:v0 + 256], V[:, v0:v0 + 256],
                   V[:, v0:v0 + 256], 256, f"ss{tag}")

        def gather0(u):
            og = pwork.tile([128, 256], F32, name=f"og0{u}", tag="og0",
                            bufs=2)
            nc.gpsimd.ap_gather(
                out_ap=og[:], in_ap=T0[u][:],
                idxs_ap=widxB[:, 64 + u * 16:64 + (u + 1) * 16],
                channels=128, num_elems=8192, d=1, num_idxs=256,
            )
            for sec in range(2):
                nc.vector.tensor_tensor(
                    out=og[:, sec * 128:(sec + 1) * 128],
                    in0=og[:, sec * 128:(sec + 1) * 128],
                    in1=wb[:, 1024 + u * 128:1024 + (u + 1) * 128],
                    op=AL.mult,
                )
            v0 = 512 + u * 64
            nc.vector.tensor_reduce(
                out=V[:, v0:v0 + 64],
                in_=og[:].rearrange("c (n f) -> c n f", f=4),
                axis=mybir.AxisListType.X, op=AL.add,
            )
            v0u = V[:, v0:v0 + 64]
            colsum(ps_ss0[:, u * 64:(u + 1) * 64], v0u, v0u, 64, f"ss0{u}")

        def dots0(u):
            # cross-level dots for image u (needs V1/V2 slices emitted)
            v0u = V[:, 512 + 64 * u:512 + 64 * (u + 1)]
            v1u = V[:, 256:512].rearrange(
                "c (sec b n) -> c sec b n", sec=2, b=BL
            )[:, :, u, :]
            v2u = V[:, 0:256].rearrange(
                "c (sec b n) -> c sec b n", sec=2, b=BL
            )[:, :, u, :]
            sl = slice(u * 64, (u + 1) * 64)
            colsum(ps_d01[:, sl], v0u, v1u, 64, f"d01{u}")
            colsum(ps_d02[:, sl], v0u, v2u, 64, f"d02{u}")

        ssc = pool.tile([1, 384], F32, name="ssc")
        dc = pool.tile([1, 384], F32, name="dc")
        nrm = pool.tile([1, 384], F32, name="nrm")
        rn = pool.tile([1, 384], F32, name="rn")
        rp = pool.tile([1, 384], F32, name="rp")

        def secsum(dst, src, l0_layout):
            # single-input reduce over the chunk axis (PSUM-legal)
            if l0_layout:  # src [1, 256] cols (u, sec, n)
                v = src.rearrange("o (u sec n) -> o u n sec", u=BL, sec=2)
                nc.vector.tensor_reduce(
                    out=dst.rearrange("o (u n) -> o u n", u=BL),
                    in_=v, axis=mybir.AxisListType.X, op=AL.add,
                )
            else:  # src [1, 256] cols (sec, b, n)
                nc.vector.tensor_reduce(
                    out=dst,
                    in_=src.rearrange("o (sec n) -> o n sec", sec=2),
                    axis=mybir.AxisListType.X, op=AL.add,
                )

        def norm_chain(sl):
            # rn[sl] = 1/max(sqrt(ssc[sl]), EPS) == 1/sqrt(max(ssc[sl], EPS^2))
            nc.vector.tensor_scalar_max(
                out=ssc[:, sl], in0=ssc[:, sl], scalar1=EPS * EPS
            )
            nc.scalar.sqrt(out=nrm[:, sl], in_=ssc[:, sl])
            nc.vector.reciprocal(out=rn[:, sl], in_=nrm[:, sl])

        gatherL(2, T2, widx2[:], 0, 0, "2")
        gatherL(1, T1, widxB[:, 0:64], 256, 512, "1")
        colsum(ps_d12[:], V[:, 256:512], V[:, 0:256], 256, "d12")
        # l1/l2 norms + the (1,2) pair term run early, off the tail
        secsum(LSEG(ssc, 1), ps_ss[:, 256:512], False)
        secsum(LSEG(ssc, 2), ps_ss[:, 0:256], False)
        norm_chain(slice(128, 384))
        nc.vector.tensor_tensor(
            out=LSEG(rp, 2), in0=LSEG(rn, 1), in1=LSEG(rn, 2), op=AL.mult
        )
        secsum(LSEG(dc, 2), ps_d12[:], False)
        nc.vector.tensor_tensor(
            out=LSEG(dc, 2), in0=LSEG(dc, 2), in1=LSEG(rp, 2), op=AL.mult
        )
        for u in range(BL):
            gather0(u)
            dots0(u)

        # ---- tail epilogue: only the l0-dependent parts ----
        secsum(LSEG(ssc, 0), ps_ss0[:], True)
        norm_chain(slice(0, 128))
        nc.vector.tensor_tensor(
            out=LSEG(rp, 0), in0=LSEG(rn, 0), in1=LSEG(rn, 1), op=AL.mult
        )
        nc.vector.tensor_tensor(
            out=LSEG(rp, 1), in0=LSEG(rn, 0), in1=LSEG(rn, 2), op=AL.mult
        )
        secsum(LSEG(dc, 0), ps_d01[:], True)
        secsum(LSEG(dc, 1), ps_d02[:], True)
        nc.vector.tensor_tensor(
            out=dc[:, 0:256], in0=dc[:, 0:256], in1=rp[:, 0:256], op=AL.mult
        )
        res = pool.tile([1, 1], F32)
        nc.vector.tensor_reduce(
            out=res[:], in_=dc[:], axis=mybir.AxisListType.X, op=AL.add
        )
        nc.sync.dma_start(out=out.ap(), in_=res[:])

    nc.compile()
    return nc


def _get_program():
    if "nc" not in _CACHE:
        _CACHE["nc"] = _build_program()
    return _CACHE["nc"]


def _run_device(feat0, feat1, feat2, boxes, **run_kwargs):
    from concourse.bass_utils import run_bass_kernel_spmd

    nc = _get_program()

    feats = [
        np.ascontiguousarray(np.asarray(f, dtype=np.float32))
        for f in (feat0, feat1, feat2)
    ]
    boxes = np.ascontiguousarray(np.asarray(boxes, dtype=np.float32))

    in_maps = []
    for k in range(N_CORES):
        sl = slice(k * BL, (k + 1) * BL)
        in_maps.append(
            {
                "feat0": feats[0][sl],
                "feat1": feats[1][sl],
                "feat2": feats[2][sl],
                "boxes": boxes[sl],
            }
        )

    return run_bass_kernel_spmd(
        nc, in_maps, core_ids=list(range(N_CORES)), **run_kwargs
    )


def kernel(feat0, feat1, feat2, boxes):
    r = _run_device(feat0, feat1, feat2, boxes)
    total = np.float64(0.0)
    for m in r.results:
        total += np.float64(m["out"].reshape(-1)[0])

    count = B * N * len(PAIRS)
    avg = np.float32(total) / np.float32(count)
    loss = np.float32(1.0) - avg
    loss = np.nan_to_num(loss, nan=0.0, posinf=1.0, neginf=0.0)
    return np.array(np.clip(loss, 0.0, 2.0), dtype=np.float32)


# revision 20
# speedup vs baseline: 1.2561x; 1.2561x over previous
"""Trainium2 Bass kernel for nn_CSCLoss: multi-scale bilinear point-sampling
cosine-consistency loss.

loss = 1 - mean_{pairs,(b,n)} <normalize(sample(feat_i, p_bn)), normalize(sample(feat_j, p_bn))>

Sharding: data-parallel over batch - 32 images -> 8 cores x 4 images; the
host sums the 8 per-core partial sums and applies the loss epilogue.

Per-core dataflow (v3). ap_gather costs ~26ns/idx (d=2) under concurrent
streaming, so the design minimizes INDEX COUNT with d=2 pair-gathers
everywhere (1536 idx total):
 - Every level gets an [A | B] tile where B = A shifted by one element,
   built by the otherwise-idle ACT engine (nc.scalar.copy) - no HBM
   re-read, no DMA-fabric traffic. An x-pair (p, p+1) is an even-aligned
   d=2 block of A if p even, of B if p odd: one index per (point, row).
 - Queue split: streams ride the sync HWDGE queue; boxes, index
   replication, shift-copies and the result ride the scalar queue; the
   Pool queue runs nothing but ap_gather (no SWDGE ucode swaps).
 - l1's 512-idx gather is split into 4 x 128-idx pieces so l0 chunk
   gathers interleave on the Q7 queue without stalling the 2-slot T0
   rotation. l0: per-(image, chunk) tiles, one 64-idx gather each (the
   index list is chunk-independent).
 - Index math on partition 0 in wide fused DVE ops (int16 parity trick:
   idx = (base>>1) + (base&1)*HALF); replication to the 8 gpsimd core
   groups via DRAM round trips.
 - V slices in (b, sec, n) layout; per-chunk channel sums (ones-matmul
   into PSUM) right after each V slice; l1/l2 norms + the (1,2) pair run
   early; only the l0-dependent epilogue rides the tail.
"""

import sys
from contextlib import ExitStack

import numpy as np

if "/opt/trn_rl_repo" not in sys.path:
    sys.path.insert(0, "/opt/trn_rl_repo")

B, N, C = 32, 32, 256
LEVELS = [(64, 64), (32, 32), (16, 16)]  # (H, W)
N_CORES = 8
BL = B // N_CORES          # images per core
NPTS = BL * N              # 128 points per core
PAIRS = [(0, 1), (0, 2), (1, 2)]
EPS = 1e-12

_CACHE = {}


def _build_program():
    from concourse import bacc, bass, mybir, tile, library_config

    dt = mybir.dt
    AL = mybir.AluOpType
    F32 = dt.float32
    I16 = dt.int16
    I32 = dt.int32

    nc = bacc.Bacc("TRN2", target_bir_lowering=False, debug=False)

    feats = [
        nc.dram_tensor(f"feat{i}", [BL, C, H, W], F32, kind="ExternalInput")
        for i, (H, W) in enumerate(LEVELS)
    ]
    boxes = nc.dram_tensor("boxes", [BL, N, 4], F32, kind="ExternalInput")
    out = nc.dram_tensor("out", [1, 1], F32, kind="ExternalOutput")

    with tile.TileContext(nc) as tc, ExitStack() as ctx:
        pool = ctx.enter_context(tc.tile_pool(name="sbuf", bufs=1))
        pa = ctx.enter_context(tc.tile_pool(name="pa", bufs=1))
        pstream = ctx.enter_context(tc.tile_pool(name="stream", bufs=1))
        pwork = ctx.enter_context(tc.tile_pool(name="work", bufs=2))
        ppsum = ctx.enter_context(tc.tile_pool(name="psum", bufs=1, space="PSUM"))
        pdram = ctx.enter_context(tc.tile_pool(name="dram", bufs=1, space="DRAM"))

        nc.gpsimd.load_library(library_config.ap_gather)

        # warm-up: absorb the Q7 ucode install under the stream head
        dg_src = pool.tile([128, 4], F32, name="dg_src")
        nc.vector.memset(dg_src[:], 0.0)
        dg_idx = pool.tile([128, 1], I16, name="dg_idx")
        nc.vector.memset(dg_idx[:], 0)
        dg_out = pool.tile([128, 16], F32, name="dg_out")
        nc.gpsimd.ap_gather(
            out_ap=dg_out[:], in_ap=dg_src[:], idxs_ap=dg_idx[:],
            channels=128, num_elems=4, d=1, num_idxs=16,
        )

        # ---- boxes first on the scalar queue ----
        bxr = pool.tile([1, BL * N * 4], F32)
        nc.scalar.dma_start(
            out=bxr[:].rearrange("o (a f) -> o a f", a=BL),
            in_=boxes.rearrange("b n c -> b (n c)"),
        )

        # ---- stream tiles: [A | B] per level, B = A shifted one elem ----
        # T2AB: A cols b*512 + sec*256 + (y*16+x), 2048 elems; B at +2048
        # T1AB: A cols b*2048 + sec*1024 + (y*32+x), 8192; B at +8192
        # T0AB (u, sec): A cols (y*64+x), 4096; B at +4096
        # Streams ride the sync queue; shift-copies ride the scalar (ACT)
        # queue in the order T2B, [widx DMAs], T1B, c0B..c7B.
        T2 = pstream.tile([128, 4096], F32, name="T2")      # 16 KB/part
        T1 = pstream.tile([128, 16384], F32, name="T1")     # 64 KB/part
        T0 = [
            pstream.tile([128, 8192], F32, name=f"T0_{u}_{sec}", tag="T0",
                         bufs=2)
            for u in range(BL) for sec in range(2)
        ]

        def bshift(T, n):
            # B = A shifted by one element. B's last element is never
            # written NOR read: gathers use num_elems = n - 1 (pairs).
            nc.scalar.copy(out=T[:, n:2 * n - 1], in_=T[:, 1:n])

        fv2 = feats[2].rearrange("b (s c) h w -> c b s (h w)", s=2)
        nc.sync.dma_start(
            out=T2[:, 0:2048].rearrange("c (b s p) -> c b s p", s=2, b=BL),
            in_=fv2,
        )
        fv1 = feats[1].rearrange("b (s c) h w -> c b s (h w)", s=2)
        nc.sync.dma_start(
            out=T1[:, 0:8192].rearrange("c (b s p) -> c b s p", s=2, b=BL),
            in_=fv1,
        )
        fv0 = feats[0].rearrange("b (s c) h w -> b s c (h w)", s=2)
        for k in range(2 * BL):
            u, sec = k // 2, k % 2
            nc.sync.dma_start(out=T0[k][:, 0:4096], in_=fv0[u, sec])
        bshift(T2, 2048)

        # ---- constants (DVE, no deps - run under the stream head) ----
        # per-level column layout on [1, 384]: cols li*128 + (b*32 + n)
        LSEG = lambda t, li: t[:, li * 128:(li + 1) * 128]
        WVf = pa.tile([1, 384], F32, name="WVf")    # W per level (y stride)
        WVi = pa.tile([1, 384], I32, name="WVi")
        for li, (H, W) in enumerate(LEVELS):
            nc.vector.memset(LSEG(WVf, li), float(W))
            nc.vector.memset(LSEG(WVi, li), W)
        OFFV = pa.tile([1, 384], F32, name="OFFV")  # per-image tile offset
        nc.vector.memset(LSEG(OFFV, 0), 0.0)
        for li in (1, 2):
            bstride = 2 * LEVELS[li][0] * LEVELS[li][1]  # both chunks
            ov = LSEG(OFFV, li).rearrange("o (b n) -> o b n", b=BL)
            for b in range(BL):
                nc.vector.memset(ov[:, b], float(b * bstride))
        ones1 = pool.tile([1, 128], F32, name="ones1")
        nc.vector.memset(ones1[:], 1.0)
        ones = pool.tile([128, 1], F32)
        nc.vector.memset(ones[:], 1.0)

        # ---- Phase A: per-point scalar math on partition 0 (DVE) ----
        # X-layout [1, 768]: col = li*256 + ax*128 + pt   (ax: 0=x, 1=y)
        W2 = pa.tile([1, 1536], F32, name="W2")  # [0:768] doubles as scratch
        wrow = pa.tile([1, 1536], F32, name="wrow")  # [0:768] = IFX scratch
        PF = pa.tile([1, 768], F32, name="PF")
        cview = bxr[:].rearrange("o (pt c) -> o c pt", c=4)
        for li, (H, W) in enumerate(LEVELS):
            sl = slice(li * 256, (li + 1) * 256)
            pv = PF[:, sl].rearrange("o (ax pt) -> o ax pt", ax=2)
            # p = clip(c*(E-1), 0, E-1)
            nc.vector.tensor_scalar(
                out=pv, in0=cview[:, 0:2, :], scalar1=float(W - 1),
                scalar2=0.0, op0=AL.mult, op1=AL.max,
            )
            nc.vector.tensor_scalar_min(
                out=PF[:, sl], in0=PF[:, sl], scalar1=float(W - 1)
            )
        # e0 = clamp(floor(p), 0, E-2); floor via 16.16 fixed point (exact)
        PXS = W2[:, 0:768]
        IFX = wrow[:, 0:768].bitcast(I32)
        nc.vector.tensor_scalar(
            out=PXS, in0=PF[:], scalar1=65536.0, scalar2=None, op0=AL.mult
        )
        nc.vector.tensor_copy(out=IFX, in_=PXS)
        nc.vector.tensor_scalar(
            out=IFX, in0=IFX, scalar1=16, scalar2=None,
            op0=AL.arith_shift_right,
        )
        E0F = pa.tile([1, 768], F32, name="E0F")
        nc.vector.tensor_copy(out=E0F[:], in_=IFX)
        for li, (H, W) in enumerate(LEVELS):
            sl = slice(li * 256, (li + 1) * 256)
            nc.vector.tensor_scalar_min(
                out=E0F[:, sl], in0=E0F[:, sl], scalar1=float(W - 2)
            )
        # W2 [1, 2*768]: col = w*768 + li*256 + ax*128 + pt (w: 0=1-f, 1=f)
        nc.vector.tensor_tensor(
            out=W2[:, 768:1536], in0=PF[:], in1=E0F[:], op=AL.subtract
        )
        nc.vector.tensor_scalar(
            out=W2[:, 0:768], in0=W2[:, 768:1536], scalar1=-1.0, scalar2=1.0,
            op0=AL.mult, op1=AL.add,
        )
        # base = b_off + y0*W + x0  -> BI int16 [1, 384]
        E0v = E0F[:].rearrange("o (li ax pt) -> o li ax pt", li=3, ax=2)
        BF = pa.tile([1, 384], F32, name="BF")
        BFv = BF[:].rearrange("o (li pt) -> o li pt", li=3)
        nc.vector.tensor_tensor(
            out=BFv, in0=E0v[:, :, 1, :],
            in1=WVf[:].rearrange("o (li pt) -> o li pt", li=3), op=AL.mult
        )
        nc.vector.tensor_tensor(out=BFv, in0=BFv, in1=E0v[:, :, 0, :], op=AL.add)
        nc.vector.tensor_tensor(
            out=BFv, in0=BFv,
            in1=OFFV[:].rearrange("o (li pt) -> o li pt", li=3), op=AL.add
        )
        BI = pa.tile([1, 384], I32, name="BI")
        nc.vector.tensor_copy(out=BI[:], in_=BF[:])
        # BR [1, 2*384] i32: per-row bases, col = row*384 + li*128 + pt
        # (i32 chain - the int16 shift is invalid ISA; the scatter writes
        # convert to int16 on output)
        BR = pa.tile([1, 768], I32, name="BR")
        nc.vector.tensor_copy(out=BR[:, 0:384], in_=BI[:])
        nc.vector.tensor_tensor(
            out=BR[:, 384:768], in0=BI[:], in1=WVi[:], op=AL.add
        )
        # d=2 parity: idx = (base>>1) + (base&1)*HALF     [1, 768] i32
        # PAR borrows wrow's scratch region (wrow is written later)
        PAR = wrow[:, 768:1536].bitcast(I32)
        nc.vector.tensor_scalar(
            out=PAR, in0=BR[:], scalar1=1, scalar2=None, op0=AL.bitwise_and
        )
        for li, half in ((0, 2048), (1, 4096), (2, 1024)):
            pv = PAR.rearrange("o (r li n) -> o li r n", r=2, li=3)[:, li]
            nc.vector.tensor_scalar_mul(out=pv, in0=pv, scalar1=half)
        IDXD = pa.tile([1, 768], I32, name="IDXD")
        nc.vector.tensor_scalar(
            out=IDXD[:], in0=BR[:], scalar1=1, scalar2=None,
            op0=AL.arith_shift_right,
        )
        nc.vector.tensor_tensor(out=IDXD[:], in0=IDXD[:], in1=PAR, op=AL.add)

        # ---- gather index rows: idx #m at wrapped [r=m%16, q=m//16] ----
        # IDXD col (row, li, pt) = row*384 + li*128 + b*32 + nd*8 + nm
        # (pt = b*32 + n, n = nd*8 + nm)
        def idxv(li):
            return IDXD[:].rearrange(
                "o (row li b nd nm) -> o li nm row b nd",
                row=2, li=3, b=BL, nd=4, nm=8,
            )[:, li]

        # l2 (one 512-idx gather): m = b*128 + sec*64 + n*2 + row ->
        # r = nm*2+row, q = b*8 + sec*4 + nd.  srow2 flat = r*32 + q
        srow2 = pa.tile([1, 512], I16, name="srow2")
        s2v = srow2[:].rearrange(
            "o (nm row b sec nd) -> o nm row b sec nd",
            nm=8, row=2, b=BL, sec=2,
        )
        for sec in range(2):
            nc.vector.tensor_scalar_add(
                out=s2v[:, :, :, :, sec], in0=idxv(2), scalar1=sec * 128
            )
        # srowB [1, 768] (Q=48): l1 pieces at q = b*8 + sec*4 + nd (piece =
        # image b, m = sec*64 + n*2 + row); l0 at q = 32 + u*4 + nd (per
        # image u, m = n*2 + row, same list for both chunks). flat = r*48+q
        srowB = pa.tile([1, 768], I16, name="srowB")
        sBv = srowB[:].rearrange("o (nm row q) -> o nm row q", nm=8, row=2)
        l1q = sBv[:, :, :, 0:32].rearrange(
            "o nm row (b sec nd) -> o nm row b sec nd", b=BL, sec=2
        )
        for sec in range(2):
            nc.vector.tensor_scalar_add(
                out=l1q[:, :, :, :, sec], in0=idxv(1), scalar1=sec * 512
            )
        l0q = sBv[:, :, :, 32:48].rearrange(
            "o nm row (u nd) -> o nm row u nd", u=BL
        )
        nc.vector.tensor_scalar_add(out=l0q, in0=idxv(0), scalar1=0)

        # replicate rows: SBUF -> DRAM -> broadcast (scalar HWDGE)
        widx2 = pool.tile([128, 32], I16, name="widx2")
        s2d = pdram.tile([16, 32], I16, name="s2d")
        nc.scalar.dma_start(
            out=s2d[:], in_=srow2[:].rearrange("o (r q) -> o r q", r=16)
        )
        nc.scalar.dma_start(
            out=widx2[:], in_=s2d[:].unsqueeze(0).broadcast_to([8, 16, 32])
        )
        widxB = pool.tile([128, 48], I16, name="widxB")
        sBd = pdram.tile([16, 48], I16, name="sBd")
        nc.scalar.dma_start(
            out=sBd[:], in_=srowB[:].rearrange("o (r q) -> o r q", r=16)
        )
        nc.scalar.dma_start(
            out=widxB[:], in_=sBd[:].unsqueeze(0).broadcast_to([8, 16, 48])
        )
        bshift(T1, 8192)
        for k in range(2 * BL):
            bshift(T0[k], 4096)

        # ---- lerp weights wrow [1, 1536] -> wb [128, 1536] ----
        # col = w0(level) + pt*4 + k, k = row*2 + j; weight = yw(row)*xw(j)
        # level regions: l2 at 0, l1 at 512, l0 at 1024 (pt = u*32+n)
        for li, w0 in ((2, 0), (1, 512), (0, 1024)):
            wseg = wrow[:, w0:w0 + 512].rearrange(
                "o (pt row j) -> o pt row j", pt=128, row=2, j=2
            )
            for row in range(2):
                yv = W2[:, row * 768 + li * 256 + 128:row * 768 + li * 256 + 256]
                for j in range(2):
                    xv = W2[:, j * 768 + li * 256:j * 768 + li * 256 + 128]
                    nc.vector.tensor_tensor(
                        out=wseg[:, :, row, j], in0=yv, in1=xv, op=AL.mult
                    )
        wb_ps = ppsum.tile([128, 1536], F32, name="wb_ps")
        for i in range(3):
            nc.tensor.matmul(
                wb_ps[:, i * 512:(i + 1) * 512], ones1[:],
                wrow[:, i * 512:(i + 1) * 512], start=True, stop=True,
            )
        wb = pool.tile([128, 1536], F32, name="wb")
        nc.vector.tensor_copy(out=wb[:], in_=wb_ps[:])

        # ---- gathers + lerp + reduce + per-chunk channel sums ----
        V = pool.tile([128, 768], F32, name="V")

        ps_ss = ppsum.tile([1, 512], F32, name="ps_ss")    # ss2 | ss1
        ps_ss0 = ppsum.tile([1, 256], F32, name="ps_ss0")  # (u, sec, n)
        ps_d12 = ppsum.tile([1, 256], F32, name="ps_d12")
        ps_d01 = ppsum.tile([1, 256], F32, name="ps_d01")
        ps_d02 = ppsum.tile([1, 256], F32, name="ps_d02")

        def colsum(ps_slice, in0, in1, n, tag):
            prod = pwork.tile([128, 256], F32, name=f"prod{tag}", tag="prod",
                              bufs=1)
            nc.vector.tensor_tensor(
                out=prod[:, 0:n], in0=in0, in1=in1, op=AL.mult
            )
            nc.tensor.matmul(
                ps_slice, ones[:], prod[:, 0:n], start=True, stop=True
            )

        og2 = pwork.tile([128, 1024], F32, name="og2", tag="ogL", bufs=2)
        og1 = pwork.tile([128, 1024], F32, name="og1", tag="ogL", bufs=2)

        def gatherL(og, T, idxs, nelem, nidx, o0):
            # og cols (b, sec, n, row, j)
            nc.gpsimd.ap_gather(
                out_ap=og[:, o0:o0 + 2 * nidx],
                in_ap=T[:, 0:2 * nelem].rearrange("c (n e) -> c n e", e=2),
                idxs_ap=idxs, channels=128, num_elems=nelem, d=2,
                num_idxs=nidx,
            )

        def procL(og, v0, w0, tag):
            # multiply weights (per sec), reduce 4 corners, channel-sums
            ogv = og[:].rearrange("c (b sec nk) -> c b sec nk", b=BL, sec=2)
            wbv = wb[:, w0:w0 + 512].rearrange("c (b nk) -> c b nk", b=BL)
            for sec in range(2):
                nc.vector.tensor_tensor(
                    out=ogv[:, :, sec, :], in0=ogv[:, :, sec, :], in1=wbv,
                    op=AL.mult,
                )
            nc.vector.tensor_reduce(
                out=V[:, v0:v0 + 256],
                in_=og[:].rearrange("c (n f) -> c n f", f=4),
                axis=mybir.AxisListType.X, op=AL.add,
            )
            colsum(ps_ss[:, v0:v0 + 256], V[:, v0:v0 + 256],
                   V[:, v0:v0 + 256], 256, f"ss{tag}")

        def gather0(u, sec):
            og = pwork.tile([128, 128], F32, name=f"og0{u}{sec}", tag="og0",
                            bufs=2)
            nc.gpsimd.ap_gather(
                out_ap=og[:],
                in_ap=T0[2 * u + sec][:, 0:8190].rearrange(
                    "c (n e) -> c n e", e=2
                ),
                idxs_ap=widxB[:, 32 + u * 4:36 + u * 4],
                channels=128, num_elems=4095, d=2, num_idxs=64,
            )
            return og

        def proc0(og, u, sec):
            nc.vector.tensor_tensor(
                out=og[:], in0=og[:],
                in1=wb[:, 1024 + u * 128:1024 + (u + 1) * 128], op=AL.mult
            )
            v0 = 512 + u * 64 + sec * 32
            nc.vector.tensor_reduce(
                out=V[:, v0:v0 + 32],
                in_=og[:].rearrange("c (n f) -> c n f", f=4),
                axis=mybir.AxisListType.X, op=AL.add,
            )

        def ss0(u):
            v0u = V[:, 512 + u * 64:512 + (u + 1) * 64]
            colsum(ps_ss0[:, u * 64:(u + 1) * 64], v0u, v0u, 64, f"ss0{u}")

        def dots0(u):
            # cross-level dots for image u; all V slices are (b, sec, n)
            v0u = V[:, 512 + 64 * u:512 + 64 * (u + 1)]
            v1u = V[:, 256 + 64 * u:256 + 64 * (u + 1)]
            v2u = V[:, 64 * u:64 * (u + 1)]
            sl = slice(u * 64, (u + 1) * 64)
            colsum(ps_d01[:, sl], v0u, v1u, 64, f"d01{u}")
            colsum(ps_d02[:, sl], v0u, v2u, 64, f"d02{u}")

        ssc = pool.tile([1, 384], F32, name="ssc")
        dc = pool.tile([1, 384], F32, name="dc")
        nrm = pool.tile([1, 384], F32, name="nrm")
        rn = pool.tile([1, 384], F32, name="rn")
        rp = pool.tile([1, 384], F32, name="rp")

        def secsum(dst, src):
            # reduce over the chunk axis; src [1, 256] cols (b, sec, n)
            v = src.rearrange("o (u sec n) -> o u n sec", u=BL, sec=2)
            nc.vector.tensor_reduce(
                out=dst.rearrange("o (u n) -> o u n", u=BL),
                in_=v, axis=mybir.AxisListType.X, op=AL.add,
            )

        def norm_chain(sl):
            # rn[sl] = 1/max(sqrt(ssc[sl]), EPS) == 1/sqrt(max(ssc[sl], EPS^2))
            nc.vector.tensor_scalar_max(
                out=ssc[:, sl], in0=ssc[:, sl], scalar1=EPS * EPS
            )
            nc.scalar.sqrt(out=nrm[:, sl], in_=ssc[:, sl])
            nc.vector.reciprocal(out=rn[:, sl], in_=nrm[:, sl])

        # ---- Q7 queue: g2, then l0 chunks + l1 pieces interleaved ----
        gatherL(og2, T2, widx2[:], 2047, 512, 0)
        g0t = {}
        g0t[(0, 0)] = gather0(0, 0)
        g0t[(0, 1)] = gather0(0, 1)
        gatherL(og1, T1, widxB[:, 0:8], 8191, 128, 0)
        g0t[(1, 0)] = gather0(1, 0)
        gatherL(og1, T1, widxB[:, 8:16], 8191, 128, 256)
        g0t[(1, 1)] = gather0(1, 1)
        gatherL(og1, T1, widxB[:, 16:24], 8191, 128, 512)
        g0t[(2, 0)] = gather0(2, 0)
        gatherL(og1, T1, widxB[:, 24:32], 8191, 128, 768)
        g0t[(2, 1)] = gather0(2, 1)
        g0t[(3, 0)] = gather0(3, 0)
        g0t[(3, 1)] = gather0(3, 1)

        # ---- DVE processing, ordered to match expected completion ----
        procL(og2, 0, 0, "2")
        proc0(g0t[(0, 0)], 0, 0)
        proc0(g0t[(0, 1)], 0, 1)
        ss0(0)
        proc0(g0t[(1, 0)], 1, 0)
        proc0(g0t[(1, 1)], 1, 1)
        ss0(1)
        proc0(g0t[(2, 0)], 2, 0)
        # l1 (all 4 pieces landed)
        procL(og1, 256, 512, "1")
        colsum(ps_d12[:], V[:, 256:512], V[:, 0:256], 256, "d12")
        proc0(g0t[(2, 1)], 2, 1)
        ss0(2)
        proc0(g0t[(3, 0)], 3, 0)
        proc0(g0t[(3, 1)], 3, 1)
        ss0(3)
        # early epilogue off the tail (the reciprocal waits on an ACT sqrt
        # queued behind shift-copies - keep tail-critical procs above it)
        secsum(LSEG(ssc, 1), ps_ss[:, 256:512])
        secsum(LSEG(ssc, 2), ps_ss[:, 0:256])
        norm_chain(slice(128, 384))
        nc.vector.tensor_tensor(
            out=LSEG(rp, 2), in0=LSEG(rn, 1), in1=LSEG(rn, 2), op=AL.mult
        )
        secsum(LSEG(dc, 2), ps_d12[:])
        nc.vector.tensor_tensor(
            out=LSEG(dc, 2), in0=LSEG(dc, 2), in1=LSEG(rp, 2), op=AL.mult
        )
        dots0(0)
        dots0(1)
        dots0(2)
        dots0(3)

        # ---- tail epilogue: only the l0-dependent parts ----
        secsum(LSEG(ssc, 0), ps_ss0[:])
        norm_chain(slice(0, 128))
        nc.vector.tensor_tensor(
            out=LSEG(rp, 0), in0=LSEG(rn, 0), in1=LSEG(rn, 1), op=AL.mult
        )
        nc.vector.tensor_tensor(
            out=LSEG(rp, 1), in0=LSEG(rn, 0), in1=LSEG(rn, 2), op=AL.mult
        )
        secsum(LSEG(dc, 0), ps_d01[:])
        secsum(LSEG(dc, 1), ps_d02[:])
        nc.vector.tensor_tensor(
            out=dc[:, 0:256], in0=dc[:, 0:256], in1=rp[:, 0:256], op=AL.mult
        )
        res = pool.tile([1, 1], F32)
        nc.vector.tensor_reduce(
            out=res[:], in_=dc[:], axis=mybir.AxisListType.X, op=AL.add
        )
        nc.scalar.dma_start(out=out.ap(), in_=res[:])

    nc.compile()
    return nc


def _get_program():
    if "nc" not in _CACHE:
        _CACHE["nc"] = _build_program()
    return _CACHE["nc"]


def _run_device(feat0, feat1, feat2, boxes, **run_kwargs):
    from concourse.bass_utils import run_bass_kernel_spmd

    nc = _get_program()

    feats = [
        np.ascontiguousarray(np.asarray(f, dtype=np.float32))
        for f in (feat0, feat1, feat2)
    ]
    boxes = np.ascontiguousarray(np.asarray(boxes, dtype=np.float32))

    in_maps = []
    for k in range(N_CORES):
        sl = slice(k * BL, (k + 1) * BL)
        in_maps.append(
            {
                "feat0": feats[0][sl],
                "feat1": feats[1][sl],
                "feat2": feats[2][sl],
                "boxes": boxes[sl],
            }
        )

    return run_bass_kernel_spmd(
        nc, in_maps, core_ids=list(range(N_CORES)), **run_kwargs
    )


def kernel(feat0, feat1, feat2, boxes):
    r = _run_device(feat0, feat1, feat2, boxes)
    total = np.float64(0.0)
    for m in r.results:
        total += np.float64(m["out"].reshape(-1)[0])

    count = B * N * len(PAIRS)
    avg = np.float32(total) / np.float32(count)
    loss = np.float32(1.0) - avg
    loss = np.nan_to_num(loss, nan=0.0, posinf=1.0, neginf=0.0)
    return np.array(np.clip(loss, 0.0, 2.0), dtype=np.float32)


# revision 23
# speedup vs baseline: 1.4043x; 1.1179x over previous
"""Trainium2 Bass kernel for nn_CSCLoss: multi-scale bilinear point-sampling
cosine-consistency loss.

loss = 1 - mean_{pairs,(b,n)} <normalize(sample(feat_i, p_bn)), normalize(sample(feat_j, p_bn))>

Sharding: data-parallel over batch - 32 images -> 8 cores x 4 images; the
host sums the 8 per-core partial sums and applies the loss epilogue.

Per-core dataflow (v4). ap_gather costs ~27-40ns/idx (roughly independent
of d), so d=2 pair-gathers halve the cost where the shifted-B copy is
cheap, and l0 avoids any copy inside the stream-buffer rotation loop:
 - l2/l1: [A | B] tiles (B = A shifted one element, built by the idle ACT
   engine - no HBM re-read, no DMA-fabric traffic). One d=2 index per
   (point, row): 512 idx each. l1's gather is split into 4 x 128-idx
   pieces so l0 gathers interleave on the Q7 queue.
 - l0: per-(image, chunk) A-only tiles [128, 4096], 4-slot rotation, one
   128-idx d=1 4-corner gather each - the rotation loop is just
   stream -> gather, far under the 4-slot latency budget.
 - Queue split: streams ride the sync HWDGE queue; boxes, index
   replication, shift-copies, sqrt and the result ride the scalar queue;
   the Pool queue runs nothing but ap_gather (no SWDGE ucode swaps).
 - Index math on partition 0 in wide fused DVE ops (i32 chain, int16
   parity trick idx = (base>>1) + (base&1)*HALF for d=2); one combined
   DRAM round trip replicates all 1536 indices to the 8 gpsimd core
   groups.
 - V slices in (b, sec, n) layout; per-chunk channel sums (ones-matmul
   into PSUM) right after each V slice; l1/l2 norms + the (1,2) pair run
   early; only the l0-dependent epilogue rides the tail.
"""

import sys
from contextlib import ExitStack

import numpy as np

if "/opt/trn_rl_repo" not in sys.path:
    sys.path.insert(0, "/opt/trn_rl_repo")

B, N, C = 32, 32, 256
LEVELS = [(64, 64), (32, 32), (16, 16)]  # (H, W)
N_CORES = 8
BL = B // N_CORES          # images per core
NPTS = BL * N              # 128 points per core
PAIRS = [(0, 1), (0, 2), (1, 2)]
EPS = 1e-12

_CACHE = {}


def _build_program():
    from concourse import bacc, bass, mybir, tile, library_config

    dt = mybir.dt
    AL = mybir.AluOpType
    F32 = dt.float32
    I16 = dt.int16
    I32 = dt.int32

    nc = bacc.Bacc("TRN2", target_bir_lowering=False, debug=False)

    feats = [
        nc.dram_tensor(f"feat{i}", [BL, C, H, W], F32, kind="ExternalInput")
        for i, (H, W) in enumerate(LEVELS)
    ]
    boxes = nc.dram_tensor("boxes", [BL, N, 4], F32, kind="ExternalInput")
    out = nc.dram_tensor("out", [1, 1], F32, kind="ExternalOutput")

    with tile.TileContext(nc) as tc, ExitStack() as ctx:
        pool = ctx.enter_context(tc.tile_pool(name="sbuf", bufs=1))
        pa = ctx.enter_context(tc.tile_pool(name="pa", bufs=1))
        pstream = ctx.enter_context(tc.tile_pool(name="stream", bufs=1))
        pwork = ctx.enter_context(tc.tile_pool(name="work", bufs=2))
        ppsum = ctx.enter_context(tc.tile_pool(name="psum", bufs=1, space="PSUM"))
        pdram = ctx.enter_context(tc.tile_pool(name="dram", bufs=1, space="DRAM"))

        nc.gpsimd.load_library(library_config.ap_gather)

        # warm-up: absorb the Q7 ucode install under the stream head
        dg_src = pool.tile([128, 4], F32, name="dg_src")
        nc.vector.memset(dg_src[:], 0.0)
        dg_idx = pool.tile([128, 1], I16, name="dg_idx")
        nc.vector.memset(dg_idx[:], 0)
        dg_out = pool.tile([128, 16], F32, name="dg_out")
        nc.gpsimd.ap_gather(
            out_ap=dg_out[:], in_ap=dg_src[:], idxs_ap=dg_idx[:],
            channels=128, num_elems=4, d=1, num_idxs=16,
        )

        # ---- boxes first on the scalar queue ----
        bxr = pool.tile([1, BL * N * 4], F32)
        nc.scalar.dma_start(
            out=bxr[:].rearrange("o (a f) -> o a f", a=BL),
            in_=boxes.rearrange("b n c -> b (n c)"),
        )

        # ---- stream tiles ----
        # T2AB: A cols b*512 + sec*256 + (y*16+x), 2048 elems; B at +2048
        # T1AB: A cols b*2048 + sec*1024 + (y*32+x), 8192; B at +8192
        # T0 (u, sec): A-only, cols (y*64+x), [128, 4096], 4-slot rotation
        T2 = pstream.tile([128, 4096], F32, name="T2")      # 16 KB/part
        T1 = pstream.tile([128, 16384], F32, name="T1")     # 64 KB/part
        T0 = [
            pstream.tile([128, 4096], F32, name=f"T0_{u}_{sec}", tag="T0",
                         bufs=4)
            for u in range(BL) for sec in range(2)
        ]

        def bshift(T, n):
            # B = A shifted by one element. B's last element is never
            # written NOR read: gathers use num_elems = n - 1 (pairs).
            nc.scalar.copy(out=T[:, n:2 * n - 1], in_=T[:, 1:n])

        fv2 = feats[2].rearrange("b (s c) h w -> c b s (h w)", s=2)
        nc.sync.dma_start(
            out=T2[:, 0:2048].rearrange("c (b s p) -> c b s p", s=2, b=BL),
            in_=fv2,
        )
        fv1 = feats[1].rearrange("b (s c) h w -> c b s (h w)", s=2)
        nc.sync.dma_start(
            out=T1[:, 0:8192].rearrange("c (b s p) -> c b s p", s=2, b=BL),
            in_=fv1,
        )
        fv0 = feats[0].rearrange("b (s c) h w -> b s c (h w)", s=2)
        for k in range(2 * BL):
            u, sec = k // 2, k % 2
            nc.sync.dma_start(out=T0[k][:], in_=fv0[u, sec])
        bshift(T2, 2048)  # scalar queue: right after boxes

        # ---- constants (DVE, no deps - run under the stream head) ----
        # per-level column layout on [1, 384]: cols li*128 + (b*32 + n)
        LSEG = lambda t, li: t[:, li * 128:(li + 1) * 128]
        WVf = pa.tile([1, 384], F32, name="WVf")    # W per level (y stride)
        WVi = pa.tile([1, 384], I32, name="WVi")
        for li, (H, W) in enumerate(LEVELS):
            nc.vector.memset(LSEG(WVf, li), float(W))
            nc.vector.memset(LSEG(WVi, li), W)
        OFFV = pa.tile([1, 384], F32, name="OFFV")  # per-image tile offset
        nc.vector.memset(LSEG(OFFV, 0), 0.0)
        for li in (1, 2):
            bstride = 2 * LEVELS[li][0] * LEVELS[li][1]  # both chunks
            ov = LSEG(OFFV, li).rearrange("o (b n) -> o b n", b=BL)
            for b in range(BL):
                nc.vector.memset(ov[:, b], float(b * bstride))
        ones1 = pool.tile([1, 128], F32, name="ones1")
        nc.vector.memset(ones1[:], 1.0)
        ones = pool.tile([128, 1], F32)
        nc.vector.memset(ones[:], 1.0)

        # ---- Phase A: per-point scalar math on partition 0 (DVE) ----
        # X-layout [1, 768]: col = li*256 + ax*128 + pt   (ax: 0=x, 1=y)
        W2 = pa.tile([1, 1536], F32, name="W2")  # [0:768] doubles as scratch
        wrow = pa.tile([1, 1536], F32, name="wrow")  # scratch for IFX/PAR
        PF = pa.tile([1, 768], F32, name="PF")
        cview = bxr[:].rearrange("o (pt c) -> o c pt", c=4)
        for li, (H, W) in enumerate(LEVELS):
            sl = slice(li * 256, (li + 1) * 256)
            pv = PF[:, sl].rearrange("o (ax pt) -> o ax pt", ax=2)
            # p = clip(c*(E-1), 0, E-1)
            nc.vector.tensor_scalar(
                out=pv, in0=cview[:, 0:2, :], scalar1=float(W - 1),
                scalar2=0.0, op0=AL.mult, op1=AL.max,
            )
            nc.vector.tensor_scalar_min(
                out=PF[:, sl], in0=PF[:, sl], scalar1=float(W - 1)
            )
        # e0 = clamp(floor(p), 0, E-2); floor via 16.16 fixed point (exact)
        PXS = W2[:, 0:768]
        IFX = wrow[:, 0:768].bitcast(I32)
        nc.vector.tensor_scalar(
            out=PXS, in0=PF[:], scalar1=65536.0, scalar2=None, op0=AL.mult
        )
        nc.vector.tensor_copy(out=IFX, in_=PXS)
        nc.vector.tensor_scalar(
            out=IFX, in0=IFX, scalar1=16, scalar2=None,
            op0=AL.arith_shift_right,
        )
        E0F = pa.tile([1, 768], F32, name="E0F")
        nc.vector.tensor_copy(out=E0F[:], in_=IFX)
        for li, (H, W) in enumerate(LEVELS):
            sl = slice(li * 256, (li + 1) * 256)
            nc.vector.tensor_scalar_min(
                out=E0F[:, sl], in0=E0F[:, sl], scalar1=float(W - 2)
            )
        # base = b_off + y0*W + x0  -> BI i32 [1, 384]
        E0v = E0F[:].rearrange("o (li ax pt) -> o li ax pt", li=3, ax=2)
        BF = pa.tile([1, 384], F32, name="BF")
        BFv = BF[:].rearrange("o (li pt) -> o li pt", li=3)
        nc.vector.tensor_tensor(
            out=BFv, in0=E0v[:, :, 1, :],
            in1=WVf[:].rearrange("o (li pt) -> o li pt", li=3), op=AL.mult
        )
        nc.vector.tensor_tensor(out=BFv, in0=BFv, in1=E0v[:, :, 0, :], op=AL.add)
        nc.vector.tensor_tensor(
            out=BFv, in0=BFv,
            in1=OFFV[:].rearrange("o (li pt) -> o li pt", li=3), op=AL.add
        )
        BI = W2[:, 0:384].bitcast(I32)  # scratch (w1 is written later)
        nc.vector.tensor_copy(out=BI, in_=BF[:])
        # BR [1, 2*384] i32: per-row bases, col = row*384 + li*128 + pt
        BR = pa.tile([1, 768], I32, name="BR")
        nc.vector.tensor_copy(out=BR[:, 0:384], in_=BI)
        nc.vector.tensor_tensor(
            out=BR[:, 384:768], in0=BI, in1=WVi[:], op=AL.add
        )
        # d=2 parity for l2/l1: idx = (base>>1) + (base&1)*HALF   [i32]
        PAR = wrow[:, 768:1536].bitcast(I32)  # scratch (wrow written later)
        nc.vector.tensor_scalar(
            out=PAR, in0=BR[:], scalar1=1, scalar2=None, op0=AL.bitwise_and
        )
        for li, half in ((1, 4096), (2, 1024)):
            pv = PAR.rearrange("o (r li n) -> o li r n", r=2, li=3)[:, li]
            nc.vector.tensor_scalar_mul(out=pv, in0=pv, scalar1=half)
        IDXD = pa.tile([1, 768], I32, name="IDXD")
        nc.vector.tensor_scalar(
            out=IDXD[:], in0=BR[:], scalar1=1, scalar2=None,
            op0=AL.arith_shift_right,
        )
        nc.vector.tensor_tensor(out=IDXD[:], in0=IDXD[:], in1=PAR, op=AL.add)

        # ---- gather index rows, all in ONE wrapped tile [16, Q=96] ----
        # idx #m of a gather sits at [r = m%16, q0 + m//16]; flat = r*96+q.
        # q 0:32  = l2 (512 idx):  m = b*128 + sec*64 + n*2 + row
        # q 32:64 = l1 pieces (4 x 128 idx, piece=b): m = sec*64 + n*2 + row
        # q 64:96 = l0 (4 x 128 idx, per image u, shared by both chunks):
        #           m = n*4 + k  (k = row*2 + j, d=1 four-corner)
        srowA = pa.tile([1, 1536], I16, name="srowA")

        def idxv(li):
            # [o, nm8, row, b, nd4] view of IDXD at level li (n = nd*8+nm)
            return IDXD[:].rearrange(
                "o (row li b nd nm) -> o li nm row b nd",
                row=2, li=3, b=BL, nd=4, nm=8,
            )[:, li]

        sv = srowA[:].rearrange("o (nm row q) -> o nm row q", nm=8, row=2)
        # l2: r = (n%8)*2+row, q = b*8 + sec*4 + n//8
        l2q = sv[:, :, :, 0:32].rearrange(
            "o nm row (b sec nd) -> o nm row b sec nd", b=BL, sec=2
        )
        for sec in range(2):
            nc.vector.tensor_scalar_add(
                out=l2q[:, :, :, :, sec], in0=idxv(2), scalar1=sec * 128
            )
        # l1: r = (n%8)*2+row, q = 32 + b*8 + sec*4 + n//8
        l1q = sv[:, :, :, 32:64].rearrange(
            "o nm row (b sec nd) -> o nm row b sec nd", b=BL, sec=2
        )
        for sec in range(2):
            nc.vector.tensor_scalar_add(
                out=l1q[:, :, :, :, sec], in0=idxv(1), scalar1=sec * 512
            )
        # l0: r = (n%4)*4 + row*2 + j, q = 64 + u*8 + n//4; idx = BR + j
        sv0 = srowA[:].rearrange(
            "o (nm row j q) -> o nm row j q", nm=4, row=2, j=2
        )
        l0q = sv0[:, :, :, :, 64:96].rearrange(
            "o nm row j (u nd) -> o nm row j u nd", u=BL
        )
        b0v = BR[:].rearrange(
            "o (row li u nd nm) -> o li nm row u nd",
            row=2, li=3, u=BL, nd=8, nm=4,
        )[:, 0]
        for j in range(2):
            nc.vector.tensor_scalar_add(
                out=l0q[:, :, :, j], in0=b0v, scalar1=j
            )

        # replicate rows: SBUF -> DRAM -> broadcast (scalar HWDGE)
        widx = pool.tile([128, 96], I16, name="widx")
        sAd = pdram.tile([16, 96], I16, name="sAd")
        nc.scalar.dma_start(
            out=sAd[:], in_=srowA[:].rearrange("o (r q) -> o r q", r=16)
        )
        nc.scalar.dma_start(
            out=widx[:], in_=sAd[:].unsqueeze(0).broadcast_to([8, 16, 96])
        )
        bshift(T1, 8192)  # scalar queue: after the widx DMAs

        # ---- lerp weights wrow [1, 1536] -> wb [128, 1536] ----
        # col = w0(level) + pt*4 + k, k = row*2 + j; weight = yw(row)*xw(j)
        # level regions: l2 at 0, l1 at 512, l0 at 1024 (pt = u*32+n)
        nc.vector.tensor_tensor(
            out=W2[:, 768:1536], in0=PF[:], in1=E0F[:], op=AL.subtract
        )
        nc.vector.tensor_scalar(
            out=W2[:, 0:768], in0=W2[:, 768:1536], scalar1=-1.0, scalar2=1.0,
            op0=AL.mult, op1=AL.add,
        )
        for li, w0 in ((2, 0), (1, 512), (0, 1024)):
            wseg = wrow[:, w0:w0 + 512].rearrange(
                "o (pt row j) -> o pt row j", pt=128, row=2, j=2
            )
            for row in range(2):
                yv = W2[:, row * 768 + li * 256 + 128:row * 768 + li * 256 + 256]
                for j in range(2):
                    xv = W2[:, j * 768 + li * 256:j * 768 + li * 256 + 128]
                    nc.vector.tensor_tensor(
                        out=wseg[:, :, row, j], in0=yv, in1=xv, op=AL.mult
                    )
        wb_ps = ppsum.tile([128, 1536], F32, name="wb_ps")
        for i in range(3):
            nc.tensor.matmul(
                wb_ps[:, i * 512:(i + 1) * 512], ones1[:],
                wrow[:, i * 512:(i + 1) * 512], start=True, stop=True,
            )
        wb = pool.tile([128, 1536], F32, name="wb")
        nc.vector.tensor_copy(out=wb[:], in_=wb_ps[:])

        # ---- gathers + lerp + reduce + per-chunk channel sums ----
        V = pool.tile([128, 768], F32, name="V")

        ps_ss = ppsum.tile([1, 512], F32, name="ps_ss")    # ss2 | ss1
        ps_ss0 = ppsum.tile([1, 256], F32, name="ps_ss0")  # (u, sec, n)
        ps_d12 = ppsum.tile([1, 256], F32, name="ps_d12")
        ps_d01 = ppsum.tile([1, 256], F32, name="ps_d01")
        ps_d02 = ppsum.tile([1, 256], F32, name="ps_d02")

        def colsum(ps_slice, in0, in1, n, tag):
            prod = pwork.tile([128, 256], F32, name=f"prod{tag}", tag="prod",
                              bufs=1)
            nc.vector.tensor_tensor(
                out=prod[:, 0:n], in0=in0, in1=in1, op=AL.mult
            )
            nc.tensor.matmul(
                ps_slice, ones[:], prod[:, 0:n], start=True, stop=True
            )

        og2 = pwork.tile([128, 1024], F32, name="og2", tag="ogL", bufs=2)
        og1 = pwork.tile([128, 1024], F32, name="og1", tag="ogL", bufs=2)

        def gatherL(og, T, idxs, nelem, nidx, o0):
            # d=2 pair gather; og cols (b, sec, n, row, j)
            nc.gpsimd.ap_gather(
                out_ap=og[:, o0:o0 + 2 * nidx],
                in_ap=T[:, 0:2 * nelem].rearrange("c (n e) -> c n e", e=2),
                idxs_ap=idxs, channels=128, num_elems=nelem, d=2,
                num_idxs=nidx,
            )

        def procL(og, v0, w0, tag):
            # multiply weights (per sec), reduce 4 corners, channel-sums
            ogv = og[:].rearrange("c (b sec nk) -> c b sec nk", b=BL, sec=2)
            wbv = wb[:, w0:w0 + 512].rearrange("c (b nk) -> c b nk", b=BL)
            for sec in range(2):
                nc.vector.tensor_tensor(
                    out=ogv[:, :, sec, :], in0=ogv[:, :, sec, :], in1=wbv,
                    op=AL.mult,
                )
            nc.vector.tensor_reduce(
                out=V[:, v0:v0 + 256],
                in_=og[:].rearrange("c (n f) -> c n f", f=4),
                axis=mybir.AxisListType.X, op=AL.add,
            )
            colsum(ps_ss[:, v0:v0 + 256], V[:, v0:v0 + 256],
                   V[:, v0:v0 + 256], 256, f"ss{tag}")

        def gather0(u, sec):
            # d=1 four-corner gather; og cols (n, row, j)
            og = pwork.tile([128, 128], F32, name=f"og0{u}{sec}", tag="og0",
                            bufs=2)
            nc.gpsimd.ap_gather(
                out_ap=og[:], in_ap=T0[2 * u + sec][:],
                idxs_ap=widx[:, 64 + u * 8:72 + u * 8],
                channels=128, num_elems=4096, d=1, num_idxs=128,
            )
            return og

        def proc0(og, u, sec):
            nc.vector.tensor_tensor(
                out=og[:], in0=og[:],
                in1=wb[:, 1024 + u * 128:1024 + (u + 1) * 128], op=AL.mult
            )
            v0 = 512 + u * 64 + sec * 32
            nc.vector.tensor_reduce(
                out=V[:, v0:v0 + 32],
                in_=og[:].rearrange("c (n f) -> c n f", f=4),
                axis=mybir.AxisListType.X, op=AL.add,
            )

        def ss0(u):
            v0u = V[:, 512 + u * 64:512 + (u + 1) * 64]
            colsum(ps_ss0[:, u * 64:(u + 1) * 64], v0u, v0u, 64, f"ss0{u}")

        def dots0(u):
            # cross-level dots for image u; all V slices are (b, sec, n)
            v0u = V[:, 512 + 64 * u:512 + 64 * (u + 1)]
            v1u = V[:, 256 + 64 * u:256 + 64 * (u + 1)]
            v2u = V[:, 64 * u:64 * (u + 1)]
            sl = slice(u * 64, (u + 1) * 64)
            colsum(ps_d01[:, sl], v0u, v1u, 64, f"d01{u}")
            colsum(ps_d02[:, sl], v0u, v2u, 64, f"d02{u}")

        ssc = pool.tile([1, 384], F32, name="ssc")
        dc = pool.tile([1, 384], F32, name="dc")
        nrm = pool.tile([1, 384], F32, name="nrm")
        rn = pool.tile([1, 384], F32, name="rn")
        rp = pool.tile([1, 384], F32, name="rp")

        def secsum(dst, src):
            # reduce over the chunk axis; src [1, 256] cols (b, sec, n)
            v = src.rearrange("o (u sec n) -> o u n sec", u=BL, sec=2)
            nc.vector.tensor_reduce(
                out=dst.rearrange("o (u n) -> o u n", u=BL),
                in_=v, axis=mybir.AxisListType.X, op=AL.add,
            )

        def norm_chain(sl):
            # rn[sl] = 1/max(sqrt(ssc[sl]), EPS) == 1/sqrt(max(ssc[sl], EPS^2))
            nc.vector.tensor_scalar_max(
                out=ssc[:, sl], in0=ssc[:, sl], scalar1=EPS * EPS
            )
            nc.scalar.sqrt(out=nrm[:, sl], in_=ssc[:, sl])
            nc.vector.reciprocal(out=rn[:, sl], in_=nrm[:, sl])

        # ---- Q7 queue: g2, then l0 chunks + l1 pieces interleaved ----
        gatherL(og2, T2, widx[:, 0:32], 2047, 512, 0)
        g0t = {}
        g0t[(0, 0)] = gather0(0, 0)
        g0t[(0, 1)] = gather0(0, 1)
        gatherL(og1, T1, widx[:, 32:40], 8191, 128, 0)
        g0t[(1, 0)] = gather0(1, 0)
        gatherL(og1, T1, widx[:, 40:48], 8191, 128, 256)
        g0t[(1, 1)] = gather0(1, 1)
        gatherL(og1, T1, widx[:, 48:56], 8191, 128, 512)
        g0t[(2, 0)] = gather0(2, 0)
        gatherL(og1, T1, widx[:, 56:64], 8191, 128, 768)
        g0t[(2, 1)] = gather0(2, 1)
        g0t[(3, 0)] = gather0(3, 0)
        g0t[(3, 1)] = gather0(3, 1)

        # ---- DVE processing, ordered to match expected completion ----
        procL(og2, 0, 0, "2")
        proc0(g0t[(0, 0)], 0, 0)
        proc0(g0t[(0, 1)], 0, 1)
        ss0(0)
        proc0(g0t[(1, 0)], 1, 0)
        proc0(g0t[(1, 1)], 1, 1)
        ss0(1)
        proc0(g0t[(2, 0)], 2, 0)
        # l1 (all 4 pieces landed)
        procL(og1, 256, 512, "1")
        colsum(ps_d12[:], V[:, 256:512], V[:, 0:256], 256, "d12")
        proc0(g0t[(2, 1)], 2, 1)
        ss0(2)
        proc0(g0t[(3, 0)], 3, 0)
        proc0(g0t[(3, 1)], 3, 1)
        ss0(3)
        # early epilogue off the tail (the reciprocal waits on an ACT sqrt
        # behind the scalar queue - keep tail-critical procs above it)
        secsum(LSEG(ssc, 1), ps_ss[:, 256:512])
        secsum(LSEG(ssc, 2), ps_ss[:, 0:256])
        norm_chain(slice(128, 384))
        nc.vector.tensor_tensor(
            out=LSEG(rp, 2), in0=LSEG(rn, 1), in1=LSEG(rn, 2), op=AL.mult
        )
        secsum(LSEG(dc, 2), ps_d12[:])
        nc.vector.tensor_tensor(
            out=LSEG(dc, 2), in0=LSEG(dc, 2), in1=LSEG(rp, 2), op=AL.mult
        )
        dots0(0)
        dots0(1)
        dots0(2)
        dots0(3)

        # ---- tail epilogue: only the l0-dependent parts ----
        secsum(LSEG(ssc, 0), ps_ss0[:])
        norm_chain(slice(0, 128))
        nc.vector.tensor_tensor(
            out=LSEG(rp, 0), in0=LSEG(rn, 0), in1=LSEG(rn, 1), op=AL.mult
        )
        nc.vector.tensor_tensor(
            out=LSEG(rp, 1), in0=LSEG(rn, 0), in1=LSEG(rn, 2), op=AL.mult
        )
        secsum(LSEG(dc, 0), ps_d01[:])
        secsum(LSEG(dc, 1), ps_d02[:])
        nc.vector.tensor_tensor(
            out=dc[:, 0:256], in0=dc[:, 0:256], in1=rp[:, 0:256], op=AL.mult
        )
        res = pool.tile([1, 1], F32)
        nc.vector.tensor_reduce(
            out=res[:], in_=dc[:], axis=mybir.AxisListType.X, op=AL.add
        )
        nc.scalar.dma_start(out=out.ap(), in_=res[:])

    nc.compile()
    return nc


def _get_program():
    if "nc" not in _CACHE:
        _CACHE["nc"] = _build_program()
    return _CACHE["nc"]


def _run_device(feat0, feat1, feat2, boxes, **run_kwargs):
    from concourse.bass_utils import run_bass_kernel_spmd

    nc = _get_program()

    feats = [
        np.ascontiguousarray(np.asarray(f, dtype=np.float32))
        for f in (feat0, feat1, feat2)
    ]
    boxes = np.ascontiguousarray(np.asarray(boxes, dtype=np.float32))

    in_maps = []
    for k in range(N_CORES):
        sl = slice(k * BL, (k + 1) * BL)
        in_maps.append(
            {
                "feat0": feats[0][sl],
                "feat1": feats[1][sl],
                "feat2": feats[2][sl],
                "boxes": boxes[sl],
            }
        )

    return run_bass_kernel_spmd(
        nc, in_maps, core_ids=list(range(N_CORES)), **run_kwargs
    )


def kernel(feat0, feat1, feat2, boxes):
    r = _run_device(feat0, feat1, feat2, boxes)
    total = np.float64(0.0)
    for m in r.results:
        total += np.float64(m["out"].reshape(-1)[0])

    count = B * N * len(PAIRS)
    avg = np.float32(total) / np.float32(count)
    loss = np.float32(1.0) - avg
    loss = np.nan_to_num(loss, nan=0.0, posinf=1.0, neginf=0.0)
    return np.array(np.clip(loss, 0.0, 2.0), dtype=np.float32)
